# revision 3
# baseline (speedup 1.0000x reference)
"""ASPP + pixel-shuffle upsample + 1x1 project, on 8 TRN2 NeuronCores.

Strategy: data-parallel over batch (B=8 -> 1 image per core). Per core:
  - all convs as matmuls on the PE (bf16 inputs/weights, fp32 PSUM accum)
  - BN folded into conv weights/bias on host
  - 3x3 dilated convs = 9 shifted 1x1 taps accumulated in PSUM, reading a
    zero-padded (100x100) copy of the image resident in SBUF
  - interleave (pixel-shuffle) is never materialized: the 1x1 projection is
    applied per-branch and its ReLU output is written with a strided AP
    directly into the interleaved position of the output row buffer
  - output rows stream back to DRAM per 16-row block
"""

import numpy as np
import ml_dtypes

B, CIN, COUT, H = 8, 256, 128, 64
PAD = 18
HP = H + 2 * PAD  # 100
EPS = 1e-5
RATES = (6, 12, 18)
N_CORES = 8
NTAP = 28  # 1 (branch0 1x1) + 3 branches * 9 taps

_BF16 = ml_dtypes.bfloat16


def _shifts(t):
    if t == 0:
        return [(0, 0)]
    d = RATES[t - 1]
    return [((ky - 1) * d, (kx - 1) * d) for ky in range(3) for kx in range(3)]


def _tap_base(t):
    return 0 if t == 0 else 1 + 9 * (t - 1)


def build_program():
    import concourse.mybir as mybir
    import concourse.tile as tile
    from concourse import bacc

    f32, bf16 = mybir.dt.float32, mybir.dt.bfloat16
    Relu = mybir.ActivationFunctionType.Relu

    nc = bacc.Bacc("TRN2", target_bir_lowering=False, debug=False)
    xp = nc.dram_tensor("xp", [2, 128, HP * HP], bf16, kind="ExternalInput")
    wb = nc.dram_tensor("wb", [2, 128, NTAP * 128], bf16, kind="ExternalInput")
    wp = nc.dram_tensor("wp", [128, 128], bf16, kind="ExternalInput")
    bias = nc.dram_tensor("bias", [128, 5], f32, kind="ExternalInput")
    out = nc.dram_tensor("out", [128, 4 * H * H], f32, kind="ExternalOutput")

    with tile.TileContext(nc) as tc:
        with (
            tc.tile_pool(name="const", bufs=1) as cpool,
            tc.tile_pool(name="bf", bufs=3) as bfpool,
            tc.tile_pool(name="ob", bufs=2) as opool,
            tc.tile_pool(name="psA", bufs=2, space="PSUM") as psA,
            tc.tile_pool(name="psB", bufs=2, space="PSUM") as psB,
        ):
            xt = []
            for c in range(2):
                t_ = cpool.tile([128, HP * HP], bf16, tag=f"x{c}")
                nc.sync.dma_start(out=t_, in_=xp[c])
                xt.append(t_.rearrange("p (h w) -> p h w", w=HP))
            wt = []
            for c in range(2):
                t_ = cpool.tile([128, NTAP * 128], bf16, tag=f"w{c}")
                nc.sync.dma_start(out=t_, in_=wb[c])
                wt.append(t_)
            wpt = cpool.tile([128, 128], bf16, tag="wp")
            nc.sync.dma_start(out=wpt, in_=wp[:])
            bt = cpool.tile([128, 5], f32, tag="bias")
            nc.sync.dma_start(out=bt, in_=bias[:])

            for k in range(8):  # 8-row input chunks -> output rows 16k..16k+16
                ob = opool.tile([128, 16 * 2 * H], f32, tag="ob")
                ob3 = ob.rearrange("p (a b) -> p a b", b=2 * H)
                for t in range(4):
                    sh = _shifts(t)
                    ps = psA.tile([128, 512], f32, tag="ps")
                    n = len(sh) * 2
                    idx = 0
                    for ti, (sy, sx) in enumerate(sh):
                        col = (_tap_base(t) + ti) * 128
                        for c in range(2):
                            nc.tensor.matmul(
                                ps[:],
                                lhsT=wt[c][:, col : col + 128],
                                rhs=xt[c][
                                    :,
                                    PAD + sy + 8 * k : PAD + sy + 8 * k + 8,
                                    PAD + sx : PAD + sx + H,
                                ],
                                start=(idx == 0),
                                stop=(idx == n - 1),
                            )
                            idx += 1
                    bftile = bfpool.tile([128, 512], bf16, tag="bf")
                    nc.scalar.activation(bftile[:], ps[:], Relu, bias=bt[:, t : t + 1])
                    ps2 = psB.tile([128, 512], f32, tag="ps2")
                    nc.tensor.matmul(ps2[:], lhsT=wpt[:], rhs=bftile[:], start=True, stop=True)
                    r_, c_ = t // 2, t % 2
                    nc.scalar.activation(
                        ob3[:, r_::2, c_::2],
                        ps2.rearrange("p (a b) -> p a b", b=H),
                        Relu,
                        bias=bt[:, 4:5],
                    )
                nc.sync.dma_start(
                    out=out[:, k * 16 * 2 * H : (k + 1) * 16 * 2 * H], in_=ob[:]
                )
    nc.compile()
    return nc


def host_prep_weights(inputs):
    f32 = np.float32
    scales, biases = [], []
    for t in ("0", "1", "2", "3", "p"):
        g = np.asarray(inputs[f"g{t}"], f32)
        b = np.asarray(inputs[f"b{t}"], f32)
        m = np.asarray(inputs[f"m{t}"], f32)
        v = np.asarray(inputs[f"v{t}"], f32)
        s = g / np.sqrt(v + EPS)
        scales.append(s)
        biases.append((b - m * s).astype(f32))
    bias_arr = np.stack(biases, axis=1).astype(f32)  # (128, 5)

    wtaps = np.zeros((NTAP, CIN, COUT), f32)  # [tap, ci, co]
    w0 = np.asarray(inputs["w0"], f32)[:, :, 0, 0] * scales[0][:, None]  # (co, ci)
    wtaps[0] = w0.T
    blk = 1
    for bi, key in enumerate(("w1", "w2", "w3")):
        w = np.asarray(inputs[key], f32) * scales[bi + 1][:, None, None, None]
        for ky in range(3):
            for kx in range(3):
                wtaps[blk] = w[:, :, ky, kx].T
                blk += 1
    wb = (
        wtaps.reshape(NTAP, 2, 128, COUT)
        .transpose(1, 2, 0, 3)
        .reshape(2, 128, NTAP * COUT)
        .astype(_BF16)
    )
    wpT = (
        (np.asarray(inputs["wp"], f32)[:, :, 0, 0] * scales[4][:, None])
        .T.astype(_BF16)
        .copy()
    )
    return wb, wpT, bias_arr


def host_prep_x(x):
    x = np.asarray(x, np.float32).reshape(B, 2, 128, H, H)
    xpad = np.zeros((B, 2, 128, HP, HP), _BF16)
    xpad[:, :, :, PAD : PAD + H, PAD : PAD + H] = x
    return xpad.reshape(B, 2, 128, HP * HP)


def make_in_maps(inputs):
    wb, wpT, bias_arr = host_prep_weights(inputs)
    xpad = host_prep_x(inputs["x"])
    return [
        {"xp": xpad[b], "wb": wb, "wp": wpT, "bias": bias_arr} for b in range(B)
    ]


_NC_CACHE = []


def kernel(**inputs):
    from concourse import bass_utils

    if not _NC_CACHE:
        _NC_CACHE.append(build_program())
    nc = _NC_CACHE[0]
    in_maps = make_in_maps(inputs)
    res = bass_utils.run_bass_kernel_spmd(nc, in_maps, core_ids=list(range(N_CORES)))
    return np.stack(
        [r["out"].reshape(COUT, 2 * H, 2 * H) for r in res.results]
    ).astype(np.float32)
